# revision 63
# baseline (speedup 1.0000x reference)
"""Depth-masked 3-branch 3x3 conv (Conv2.5D) on 8 TRN2 NeuronCores.

Data-parallel over N=8 images (1 image/core). Per core:
  - x is host-prepped into two bf16 slabs tA=[x; x+1], tB=[x; x+128]
    (zero-padded), DMA'd straight to SBUF: no on-chip staging/conversion;
    depth arrives as 3 row-shifted planes + host fx/depth reciprocal
  - phi in {0,1,2,3} encodes the active branch per (tap,pixel); computed
    per tap-pair in f32 on DVE (p1 first: it heads the Pool stream),
    collapsed to 8 pixel-major rc rows (one per tap)
  - per-superchunk phi broadcast (row -> 64 partitions, both taps of a
    pair) is one SBUF->SBUF DMA with a 0-stride middle dim: no PE/Act cost
  - pair bases (each pair = 3 GEMM groups, 13 total, 52 matmuls/super):
      p0:       {phi*x, E2*x, E3*x}          E_j = (phi >= j-.5) via DVE
                                             tensor_scalar (4x mode)
      p1,p2,p3: {phi*x, r2*x, r3*x}          r_j = relu(2*phi - (2j-1)),
                                             {0,0,1,3}/{0,0,0,1} at
                                             phi=1,2,3, built by Act
    (p1 mixes: r2 from Act, E3 from DVE; all bases vanish at phi=0)
  - multiply split: phi*x of p1,p2,p3 on Pool (2127ns/1024px each), the
    other 9 on DVE (564ns) - the exact DVE/Pool equilibrium
  - out accumulated in PSUM, Act-copied to bf16 SBUF (emission deferred
    one super so it never delays Act's planes), host casts to f32
"""

import sys

sys.path.insert(0, "/opt/trn_rl_repo")

import numpy as np
import ml_dtypes

import concourse.bass as bass
import concourse.mybir as mybir
from concourse.bass_utils import run_bass_kernel_spmd
from concourse import tile
from concourse.vector_clock import VectorClock, ScopedClock

F32 = mybir.dt.float32
BF16 = mybir.dt.bfloat16
AF = mybir.ActivationFunctionType
ALU = mybir.AluOpType

N_IMG, C, O, H, W = 8, 64, 64, 128, 128
L = H * W
CHUNK = 2048
NCHUNK = L // CHUNK
BASE = 144  # pad on each side of the x slabs (window reads span +-129)
XW = BASE + L + BASE
# tap k = 3*(dh+1)+(dw+1); flat pixel offset dh*W+dw
OFF = [(k // 3 - 1) * W + (k % 3 - 1) for k in range(9)]
# tap pairs (ka,kb): off(kb)-off(ka) == 1 -> tA, == 128 -> tB
PAIRS = [(0, 1), (7, 8), (2, 5), (3, 6)]
PAIR_SRC = ["A", "A", "B", "B"]
NGRP = 13  # center + 3 per pair


def _patched_drain_and_barrier(self, tick_clock, wait_clock):
    # stock version puts every live sem wait on one drain -> walrus
    # "Too many sync wait commands"; emit one single-wait NOP per sem.
    ticks = list(tick_clock.global_clock)
    n = len(ticks)
    for i, t in enumerate(ticks):
        if t > 0:
            vec = [0] * n
            vec[i] = t
            nop = self.nc.sync.nop()
            wait_clock.add_sem_waits(nop.ins, ScopedClock({None: VectorClock(vec)}))
    self.nc.sync.drain()
    self.nc.all_engine_barrier()
    popped = self.nc._tile_sem_poison_stack.pop()
    assert popped is self._sem_poison
    self.nc.clear_and_free_semaphores(list(self.sems.allocated().values()))
    self.nc.all_engine_barrier()


tile.TileContext._drain_and_barrier = _patched_drain_and_barrier


def _split_excess_waits(nc, noop_cls, max_waits=1):
    # this walrus build rejects >1 sync-wait on several instruction
    # structs; hoist extras onto same-engine NoOps placed just before.
    for fn in nc.m.functions:
        for blk in fn.blocks:
            idx = 0
            while idx < len(blk.instructions):
                inst = blk.instructions[idx]
                si = inst.sync_info
                if si is not None and len(si.on_wait) > max_waits:
                    waits = list(si.on_wait)
                    si.on_wait = waits[-max_waits:]
                    pos = idx
                    for w in waits[:-max_waits]:
                        nop = noop_cls(
                            name=nc.get_next_instruction_name(), ins=[], outs=[]
                        )
                        nop.engine = inst.engine
                        nop.sync_info = mybir.SyncInfo(on_wait=[w], on_update=[])
                        nc.register_instruction(nop)
                        blk.instructions.insert(pos, nop)
                        pos += 1
                        idx += 1
                idx += 1


def _build_graph():
    nc = bass.Bass()
    xt_d = nc.declare_dram_parameter("xt", [128, 2 * XW], BF16, isOutput=False)
    # host-packed: cols 0:390 = dsh (3 shifted depth copies, zero-edged),
    # col 390 = 1/fx per partition
    dsh_d = nc.declare_dram_parameter("dshp", [128, 518], F32, isOutput=False)
    wp_d = nc.declare_dram_parameter("wp", [128, NGRP * 64], BF16, isOutput=False)
    out_d = nc.declare_dram_parameter("out", [O, L], BF16, isOutput=True)

    with tile.TileContext(nc) as tc:
        WIN = CHUNK + 264  # slab window: [BASE+c0-132, BASE+c0+CHUNK+132)
        with (
            tc.tile_pool(name="big", bufs=1) as big,
            tc.tile_pool(name="slab", bufs=3) as slb,
            tc.tile_pool(name="mask", bufs=1) as mk,
            tc.tile_pool(name="scr", bufs=3) as scr,
            tc.tile_pool(name="rrep", bufs=10) as rrp,
            tc.tile_pool(name="xm", bufs=10) as xmp,
            tc.tile_pool(name="sgn", bufs=6) as sgp,
            tc.tile_pool(name="ebp", bufs=3) as ebp,
            tc.tile_pool(name="outp", bufs=2) as outp,
            tc.tile_pool(name="psum", bufs=2, space=bass.MemorySpace.PSUM) as psp,
        ):
            # ---- x slabs: host-prepped bf16, zero-padded; tA rows 0:64 =
            # x, 64:128 = x shifted +1; tB rows 64:128 = x shifted +128.
            # Per-superchunk windows in a ring: tile cols [0,WIN) = tA
            # window, [WIN,2*WIN) = tB window; col 132 = global BASE+c0 ----
            def load_slabs(ci):
                # two half-DMAs: short DMA_ENGINES holds let the small
                # latency-critical collapse/bcast transfers interleave
                g0 = BASE + ci * CHUNK - 132
                xw = slb.tile([128, 2 * WIN], BF16, tag="xw")
                nc.sync.dma_start(
                    xw[:, 0:WIN],
                    bass.AP(xt_d[:].tensor, xt_d[:].offset + g0,
                            [list(xt_d[:].ap[0]), [1, WIN]]),
                )
                nc.sync.dma_start(
                    xw[:, WIN : 2 * WIN],
                    bass.AP(xt_d[:].tensor, xt_d[:].offset + XW + g0,
                            [list(xt_d[:].ap[0]), [1, WIN]]),
                )
                return xw

            # depth DMA first: it heads the critical path (phi encode)
            dshp = mk.tile([128, 518], F32)
            nc.sync.dma_start(dshp[:], dsh_d[:])

            wp = big.tile([128, NGRP * 64], BF16)
            nc.scalar.dma_start(wp[:], wp_d[:])
            slab_tiles = {0: load_slabs(0)}
            # per-partition bias vectors for the AF.Relu basis planes:
            # relu(2*phi-3) = {0,0,1,3}, relu(2*phi-5) = {0,0,0,1}
            b3 = mk.tile([128, 1], F32)
            nc.vector.memset(b3[:], -3.0)
            b5 = mk.tile([128, 1], F32)
            nc.vector.memset(b5[:], -5.0)

            # ---- depth -> phi encoding, split into two 4-tap halves so
            # pair p0/p1 broadcasts start while p2/p3 still encode.
            # Half A = taps (0,1,7,8) = pairs p0,p1: dsh offsets
            # {0,1,261,262}; half B = taps (2,5,3,6) = pairs p2,p3:
            # offsets {2,132,130,260} ----
            def _win(base, offset, dims):
                return bass.AP(
                    base.tensor, offset, [list(base.ap[0])] + [list(d) for d in dims]
                )

            rc = big.tile([8, L], BF16)
            rgb2 = _win(dshp[:], 390, [(0, 2), (1, 128)])
            cent2 = _win(dshp[:], 131, [(0, 2), (1, 128)])

            def encode_quarter(off, stride, r0):
                # one pair's two taps: dsh offsets {off, off+stride}
                dcol = _win(dshp[:], off, [(stride, 2), (1, 128)])
                et = scr.tile([128, 256], F32, tag="u")
                nc.vector.tensor_tensor(et[:], dcol, cent2, ALU.subtract)
                tq = scr.tile([128, 256], F32, tag="t")
                nc.vector.tensor_tensor(tq[:], et[:], rgb2, ALU.mult)
                ua = scr.tile([128, 256], F32, tag="u")
                nc.vector.tensor_scalar(ua[:], tq[:], -1.5, None, ALU.is_ge)
                ub = scr.tile([128, 256], F32, tag="u")
                nc.vector.scalar_tensor_tensor(
                    ub[:], tq[:], -0.5, ua[:], ALU.is_ge, ALU.add
                )
                uc = scr.tile([128, 256], F32, tag="u")
                nc.vector.scalar_tensor_tensor(
                    uc[:], tq[:], 0.5, ub[:], ALU.is_ge, ALU.add
                )
                renc = scr.tile([128, 256], BF16, tag="r")
                nc.vector.scalar_tensor_tensor(
                    renc[:], tq[:], 1.5, uc[:], ALU.is_lt, ALU.mult
                )
                for i in (0, 1):
                    eng = nc.sync if i % 2 == 0 else nc.scalar
                    eng.dma_start(
                        rc[r0 + i : r0 + i + 1, :],
                        renc[:, i * 128 : (i + 1) * 128],
                    )

            # phi broadcast for superchunk ci, pairs [plo,phi): one DMA per
            # pair; row 2p -> partitions 0:64, row 2p+1 -> 64:128
            def bcast(c0, clen=CHUNK, plo=0, phi=4, rrs=None):
                rrs = rrs if rrs is not None else [None] * 4
                for p in range(plo, phi):
                    rr = rrp.tile([128, CHUNK], BF16, tag="rr")
                    b = rc[2 * p : 2 * p + 2, c0 : c0 + clen]
                    src = bass.AP(
                        b.tensor, b.offset,
                        [list(b.ap[0]), [0, 64], [1, clen]],
                    )
                    eng = nc.sync if p % 2 == 0 else nc.scalar
                    eng.dma_start(rr[:, 0:clen], src)
                    rrs[p] = rr
                return rrs

            # p1 first: Pool's first multiply (phi*x of p1) heads the
            # serial Pool stream that paces the whole kernel
            encode_quarter(261, 1, 2)
            pipe = bcast(0, CHUNK, 1, 2)
            encode_quarter(0, 1, 0)
            bcast(0, CHUNK, 0, 1, pipe)
            encode_quarter(2, 130, 4)
            bcast(0, CHUNK, 2, 3, pipe)
            encode_quarter(130, 130, 6)
            bcast(0, CHUNK, 3, 4, pipe)
            slab_tiles[1] = load_slabs(1)

            # logical groups: 0=center; pair p slots 1+3p..3+3p = {f1,f2,f3}
            # f1 = phi*x; p0,p1: f2,f3 = E2*x, E3*x (DVE tensor_scalar);
            # p2,p3: f2,f3 = relu(2phi-3)*x, relu(2phi-5)*x (Act planes)
            # chunk plan: 7 full superchunks + 2 half chunks at the end
            # (halves shrink the post-last-build pipeline tail)
            chunks = [(i * CHUNK, CHUNK) for i in range(NCHUNK - 1)]
            b0 = (NCHUNK - 1) * CHUNK
            chunks += [(b0, 1024), (b0 + 1024, 512), (b0 + 1536, 512)]
            NITER = len(chunks)
            pending = None
            for ci in range(NITER):
                c0, clen = chunks[ci]
                reg = c0 // CHUNK
                rr = pipe
                xw = slab_tiles[reg]
                if ci + 1 < NITER:
                    pipe = bcast(*chunks[ci + 1])
                nreg = reg + 2
                if nreg < NCHUNK and nreg not in slab_tiles:
                    slab_tiles[nreg] = load_slabs(nreg)

                def xwin_of(p):
                    off = (0 if PAIR_SRC[p] == "A" else WIN) + 132
                    off += OFF[PAIRS[p][0]] + (c0 - reg * CHUNK)
                    return xw[:, off : off + clen]

                xms = {}

                # ---- Pool: phi*x for p1, p2, p3 (slow; consumed last) ----
                pool_ps = (1, 2, 3)
                for p in pool_ps:
                    xm = xmp.tile([128, CHUNK], BF16, tag="xm")
                    nc.gpsimd.tensor_tensor(
                        xm[:, 0:clen], rr[p][:, 0:clen], xwin_of(p), ALU.mult
                    )
                    xms[1 + 3 * p] = xm

                # ---- Act: relu basis planes: (p,j) in (1,2),(2,2),(2,3),
                # (3,2),(3,3); p1 f3 stays an E3 plane on DVE ----
                sg = {}
                for p, j, bv in (
                    (1, 2, b3), (2, 2, b3), (2, 3, b5), (3, 2, b3), (3, 3, b5)
                ):
                    s = sgp.tile([128, CHUNK], BF16, tag="sg")
                    nc.scalar.activation(
                        s[:, 0:clen], rr[p][:, 0:clen], AF.Relu,
                        bias=bv[:], scale=2.0,
                    )
                    sg[(p, j)] = s

                # previous chunk's PSUM -> SBUF copy + store, emitted after
                # this chunk's Act planes so it never delays them
                if pending is not None:
                    pacc, pc0, plen = pending
                    osb = outp.tile([O, CHUNK], BF16, tag="osb")
                    nc.scalar.activation(osb[:, 0:plen], pacc[:, 0:plen], AF.Copy)
                    nc.sync.dma_start(out_d[:, pc0 : pc0 + plen], osb[:, 0:plen])

                # ---- DVE: E planes + 9 multiplies ----
                def dve_tt(gid, a, p):
                    xm = xmp.tile([128, CHUNK], BF16, tag="xm")
                    nc.vector.tensor_tensor(
                        xm[:, 0:clen], a[:, 0:clen], xwin_of(p), ALU.mult
                    )
                    xms[gid] = xm

                # E planes first: DVE then streams multiplies gap-free at
                # a rate above PE consumption, so PE keeps a backlog
                ebs = {}
                for p, j in ((0, 2), (0, 3), (1, 3)):
                    eb = ebp.tile([128, CHUNK], BF16, tag="eb")
                    nc.vector.tensor_scalar(
                        eb[:, 0:clen], rr[p][:, 0:clen], j - 0.5, None, ALU.is_ge
                    )
                    ebs[(p, j)] = eb
                dve_tt(1, rr[0], 0)  # p0 f1 = phi*x
                dve_tt(2, ebs[(0, 2)], 0)
                dve_tt(3, ebs[(0, 3)], 0)
                dve_tt(6, ebs[(1, 3)], 1)
                dve_tt(5, sg[(1, 2)], 1)
                dve_tt(8, sg[(2, 2)], 2)
                dve_tt(9, sg[(2, 3)], 2)
                dve_tt(11, sg[(3, 2)], 3)
                dve_tt(12, sg[(3, 3)], 3)


                # ---- matmuls: 13 groups x PSUM banks ----
                # order: center first (free rhs), DVE tiles in build order,
                # Pool tiles last
                acc = psp.tile([O, CHUNK], F32)
                mm_rhs = [(0, xw[:, 132 + (c0 - reg * CHUNK) :
                               132 + (c0 - reg * CHUNK) + clen])]
                for gid in (1, 2, 3, 6, 5, 8, 9, 4, 11, 12, 7, 10):
                    mm_rhs.append((gid, xms[gid][:, 0:clen]))

                nmm = len(mm_rhs)
                MMN = 512
                for oi, (gid, rhs) in enumerate(mm_rhs):
                    for h in range(clen // MMN):
                        nc.tensor.matmul(
                            acc[:, h * MMN : (h + 1) * MMN],
                            wp[:, gid * 64 : (gid + 1) * 64],
                            bass.AP(
                                rhs.tensor,
                                rhs.offset + h * MMN,
                                [list(rhs.ap[0])] + [[1, MMN]],
                            ),
                            start=(oi == 0),
                            stop=(oi == nmm - 1),
                        )
                pending = (acc, c0, clen)

            pacc, pc0, plen = pending
            osb = outp.tile([O, CHUNK], BF16, tag="osb")
            nc.scalar.activation(osb[:, 0:plen], pacc[:, 0:plen], AF.Copy)
            nc.sync.dma_start(out_d[:, pc0 : pc0 + plen], osb[:, 0:plen])

    noop_cls = type(nc.sync.nop().ins)
    _split_excess_waits(nc, noop_cls, max_waits=1)
    return nc


def _bf(a):
    return a.astype(ml_dtypes.bfloat16).astype(np.float32)


def _prep_weights(w0, w1, w2):
    # basis-transformed weights; see module docstring.
    # p0 E-basis {phi,E2,E3}: V = [W2, W1-2W2, W0-W1-W2]
    # p1-p3 relu-basis {phi, relu(2phi-3), relu(2phi-5)} with values
    # {(1,2,3),(0,1,3),(0,0,1)} at phi=1,2,3: R = [W2, W1-2W2,
    # W0-3W1+3W2]  (p1 substitutes E3 for relu(2phi-5); same values)
    ws = [w0.reshape(O, C, 9), w1.reshape(O, C, 9), w2.reshape(O, C, 9)]
    W0, W1, W2 = ws
    wp = np.zeros((128, NGRP * 64), dtype=np.float32)

    def put(gi, ka, kb, arr):
        wp[0:64, gi * 64 : (gi + 1) * 64] = arr[:, :, ka].T
        if kb is not None:
            wp[64:128, gi * 64 : (gi + 1) * 64] = arr[:, :, kb].T

    put(0, 4, None, W1)  # center
    # E-basis (p0,p1) over {phi, E2, E3}; relu-basis (p2,p3) over
    # {phi, relu(2phi-3), relu(2phi-5)} = {(1,2,3),(0,1,3),(0,0,1)} at
    # phi=1,2,3 -> R = [W2, W1-2W2, W0-3W1+3W2]
    V = [W2, W1 - 2 * W2, W0 - W1 - W2]
    R = [W2, W1 - 2 * W2, W0 - 3 * W1 + 3 * W2]
    for p in range(4):
        ka, kb = PAIRS[p]
        basis = V if p == 0 else R
        for j in range(3):
            put(1 + p * 3 + j, ka, kb, basis[j])
    return wp.astype(ml_dtypes.bfloat16)


def _prep_dsh(dep, fxi):
    # [128, 518] f32: 3 x 130-col blocks of row-shifted depth (dh=-1,0,+1,
    # zero edges, 1-col left pad within each block) + cols 390:518 = fx/d
    d = np.zeros((128, 518), dtype=np.float32)
    d[1:128, 1:129] = dep[0:127]
    d[:, 131:259] = dep
    d[0:127, 261:389] = dep[1:128]
    d[:, 390:518] = fxi / dep
    return d


def _prep_x(x):
    # x: [C, L] f32 -> [128, 2*XW] bf16: tA = [x; x+1], tB = [x; x+128]
    xp = np.zeros((C, XW + 130), dtype=np.float32)
    xp[:, BASE : BASE + L] = x
    xt = np.zeros((128, 2 * XW), dtype=np.float32)
    xt[0:64, 0:XW] = xp[:, 0:XW]
    xt[64:128, 0:XW] = xp[:, 1 : XW + 1]
    xt[0:64, XW:] = xp[:, 0:XW]
    xt[64:128, XW:] = xp[:, 128 : XW + 128]
    return xt.astype(ml_dtypes.bfloat16)


def kernel(x, depth, fx, weight_0, weight_1, weight_2, _trace=False):
    x = np.asarray(x, dtype=np.float32)
    depth = np.asarray(depth, dtype=np.float32)
    fx = np.asarray(fx, dtype=np.float32)
    wp = _prep_weights(
        np.asarray(weight_0, np.float32),
        np.asarray(weight_1, np.float32),
        np.asarray(weight_2, np.float32),
    )
    in_maps = []
    for i in range(N_IMG):
        in_maps.append(
            {
                "xt": _prep_x(x[i].reshape(C, L)),
                "dshp": _prep_dsh(depth[i, 0], fx[i]),
                "wp": wp,
            }
        )
    nc = _build_graph()
    res = run_bass_kernel_spmd(nc, in_maps, core_ids=list(range(N_IMG)), trace=_trace)
    out = np.stack(
        [
            res.results[i]["out"].astype(np.float32).reshape(O, H, W)
            for i in range(N_IMG)
        ]
    )
    if _trace:
        return out, res
    return out


if __name__ == "__main__":
    rng = np.random.default_rng(0)
    ins = {
        "x": rng.standard_normal((N_IMG, C, H, W), dtype=np.float32),
        "depth": (1.0 + 9.0 * rng.random((N_IMG, 1, H, W))).astype(np.float32),
        "fx": (400.0 + 200.0 * rng.random(N_IMG)).astype(np.float32),
        "weight_0": rng.standard_normal((O, C, 3, 3), dtype=np.float32) * 0.04,
        "weight_1": rng.standard_normal((O, C, 3, 3), dtype=np.float32) * 0.04,
        "weight_2": rng.standard_normal((O, C, 3, 3), dtype=np.float32) * 0.04,
    }
    out = kernel(**ins)
    print("ran ok", out.shape, out.dtype)


# revision 64
# speedup vs baseline: 1.0048x; 1.0048x over previous
"""Depth-masked 3-branch 3x3 conv (Conv2.5D) on 8 TRN2 NeuronCores.

Data-parallel over N=8 images (1 image/core). Per core:
  - x is host-prepped into two bf16 slabs tA=[x; x+1], tB=[x; x+128]
    (zero-padded), DMA'd straight to SBUF: no on-chip staging/conversion;
    depth arrives as 3 row-shifted planes + host fx/depth reciprocal
  - phi in {0,1,2,3} encodes the active branch per (tap,pixel); computed
    per tap-pair in f32 on DVE (p1 first: it heads the Pool stream),
    collapsed to 8 pixel-major rc rows (one per tap)
  - per-superchunk phi broadcast (row -> 64 partitions, both taps of a
    pair) is one SBUF->SBUF DMA with a 0-stride middle dim: no PE/Act cost
  - pair bases (each pair = 3 GEMM groups, 13 total, 52 matmuls/super):
      p0:       {phi*x, E2*x, E3*x}          E_j = (phi >= j-.5) via DVE
                                             tensor_scalar (4x mode)
      p1,p2,p3: {phi*x, r2*x, r3*x}          r_j = relu(2*phi - (2j-1)),
                                             {0,0,1,3}/{0,0,0,1} at
                                             phi=1,2,3, built by Act
    (p1 mixes: r2 from Act, E3 from DVE; all bases vanish at phi=0)
  - multiply split: phi*x of p1,p2,p3 on Pool (2127ns/1024px each), the
    other 9 on DVE (564ns) - the exact DVE/Pool equilibrium
  - out accumulated in PSUM, Act-copied to bf16 SBUF (emission deferred
    one super so it never delays Act's planes), host casts to f32
"""

import sys

sys.path.insert(0, "/opt/trn_rl_repo")

import numpy as np
import ml_dtypes

import concourse.bass as bass
import concourse.mybir as mybir
from concourse.bass_utils import run_bass_kernel_spmd
from concourse import tile
from concourse.vector_clock import VectorClock, ScopedClock

F32 = mybir.dt.float32
BF16 = mybir.dt.bfloat16
AF = mybir.ActivationFunctionType
ALU = mybir.AluOpType

N_IMG, C, O, H, W = 8, 64, 64, 128, 128
L = H * W
CHUNK = 2048
NCHUNK = L // CHUNK
BASE = 144  # pad on each side of the x slabs (window reads span +-129)
XW = BASE + L + BASE
# tap k = 3*(dh+1)+(dw+1); flat pixel offset dh*W+dw
OFF = [(k // 3 - 1) * W + (k % 3 - 1) for k in range(9)]
# tap pairs (ka,kb): off(kb)-off(ka) == 1 -> tA, == 128 -> tB
PAIRS = [(0, 1), (7, 8), (2, 5), (3, 6)]
PAIR_SRC = ["A", "A", "B", "B"]
NGRP = 13  # center + 3 per pair


def _patched_drain_and_barrier(self, tick_clock, wait_clock):
    # stock version puts every live sem wait on one drain -> walrus
    # "Too many sync wait commands"; emit one single-wait NOP per sem.
    ticks = list(tick_clock.global_clock)
    n = len(ticks)
    for i, t in enumerate(ticks):
        if t > 0:
            vec = [0] * n
            vec[i] = t
            nop = self.nc.sync.nop()
            wait_clock.add_sem_waits(nop.ins, ScopedClock({None: VectorClock(vec)}))
    self.nc.sync.drain()
    self.nc.all_engine_barrier()
    popped = self.nc._tile_sem_poison_stack.pop()
    assert popped is self._sem_poison
    self.nc.clear_and_free_semaphores(list(self.sems.allocated().values()))
    self.nc.all_engine_barrier()


tile.TileContext._drain_and_barrier = _patched_drain_and_barrier


def _split_excess_waits(nc, noop_cls, max_waits=1):
    # this walrus build rejects >1 sync-wait on several instruction
    # structs; hoist extras onto same-engine NoOps placed just before.
    for fn in nc.m.functions:
        for blk in fn.blocks:
            idx = 0
            while idx < len(blk.instructions):
                inst = blk.instructions[idx]
                si = inst.sync_info
                if si is not None and len(si.on_wait) > max_waits:
                    waits = list(si.on_wait)
                    si.on_wait = waits[-max_waits:]
                    pos = idx
                    for w in waits[:-max_waits]:
                        nop = noop_cls(
                            name=nc.get_next_instruction_name(), ins=[], outs=[]
                        )
                        nop.engine = inst.engine
                        nop.sync_info = mybir.SyncInfo(on_wait=[w], on_update=[])
                        nc.register_instruction(nop)
                        blk.instructions.insert(pos, nop)
                        pos += 1
                        idx += 1
                idx += 1


def _build_graph():
    nc = bass.Bass()
    xt_d = nc.declare_dram_parameter("xt", [128, 2 * XW], BF16, isOutput=False)
    # host-packed: cols 0:390 = dsh (3 shifted depth copies, zero-edged),
    # col 390 = 1/fx per partition
    dsh_d = nc.declare_dram_parameter("dshp", [128, 518], F32, isOutput=False)
    wp_d = nc.declare_dram_parameter("wp", [128, NGRP * 64], BF16, isOutput=False)
    out_d = nc.declare_dram_parameter("out", [O, L], BF16, isOutput=True)

    with tile.TileContext(nc) as tc:
        WIN = CHUNK + 264  # slab window: [BASE+c0-132, BASE+c0+CHUNK+132)
        with (
            tc.tile_pool(name="big", bufs=1) as big,
            tc.tile_pool(name="slab", bufs=3) as slb,
            tc.tile_pool(name="mask", bufs=1) as mk,
            tc.tile_pool(name="scr", bufs=3) as scr,
            tc.tile_pool(name="rrep", bufs=10) as rrp,
            tc.tile_pool(name="xm", bufs=10) as xmp,
            tc.tile_pool(name="sgn", bufs=6) as sgp,
            tc.tile_pool(name="ebp", bufs=3) as ebp,
            tc.tile_pool(name="outp", bufs=2) as outp,
            tc.tile_pool(name="psum", bufs=2, space=bass.MemorySpace.PSUM) as psp,
        ):
            # ---- x slabs: host-prepped bf16, zero-padded; tA rows 0:64 =
            # x, 64:128 = x shifted +1; tB rows 64:128 = x shifted +128.
            # Per-superchunk windows in a ring: tile cols [0,WIN) = tA
            # window, [WIN,2*WIN) = tB window; col 132 = global BASE+c0 ----
            def load_slabs(ci):
                # two half-DMAs: short DMA_ENGINES holds let the small
                # latency-critical collapse/bcast transfers interleave
                g0 = BASE + ci * CHUNK - 132
                xw = slb.tile([128, 2 * WIN], BF16, tag="xw")
                nc.sync.dma_start(
                    xw[:, 0:WIN],
                    bass.AP(xt_d[:].tensor, xt_d[:].offset + g0,
                            [list(xt_d[:].ap[0]), [1, WIN]]),
                )
                nc.sync.dma_start(
                    xw[:, WIN : 2 * WIN],
                    bass.AP(xt_d[:].tensor, xt_d[:].offset + XW + g0,
                            [list(xt_d[:].ap[0]), [1, WIN]]),
                )
                return xw

            # depth DMA first: it heads the critical path (phi encode)
            dshp = mk.tile([128, 518], F32)
            nc.sync.dma_start(dshp[:], dsh_d[:])

            wp = big.tile([128, NGRP * 64], BF16)
            nc.scalar.dma_start(wp[:], wp_d[:])
            slab_tiles = {0: load_slabs(0)}
            # per-partition bias vectors for the AF.Relu basis planes:
            # relu(2*phi-3) = {0,0,1,3}, relu(2*phi-5) = {0,0,0,1}
            b3 = mk.tile([128, 1], F32)
            nc.vector.memset(b3[:], -3.0)
            b5 = mk.tile([128, 1], F32)
            nc.vector.memset(b5[:], -5.0)

            # ---- depth -> phi encoding, split into two 4-tap halves so
            # pair p0/p1 broadcasts start while p2/p3 still encode.
            # Half A = taps (0,1,7,8) = pairs p0,p1: dsh offsets
            # {0,1,261,262}; half B = taps (2,5,3,6) = pairs p2,p3:
            # offsets {2,132,130,260} ----
            def _win(base, offset, dims):
                return bass.AP(
                    base.tensor, offset, [list(base.ap[0])] + [list(d) for d in dims]
                )

            rc = big.tile([8, L], BF16)
            rgb2 = _win(dshp[:], 390, [(0, 2), (1, 128)])
            cent2 = _win(dshp[:], 131, [(0, 2), (1, 128)])

            def encode_quarter(off, stride, r0):
                # one pair's two taps: dsh offsets {off, off+stride}
                dcol = _win(dshp[:], off, [(stride, 2), (1, 128)])
                et = scr.tile([128, 256], F32, tag="u")
                nc.vector.tensor_tensor(et[:], dcol, cent2, ALU.subtract)
                tq = scr.tile([128, 256], F32, tag="t")
                nc.vector.tensor_tensor(tq[:], et[:], rgb2, ALU.mult)
                ua = scr.tile([128, 256], F32, tag="u")
                nc.vector.tensor_scalar(ua[:], tq[:], -1.5, None, ALU.is_ge)
                ub = scr.tile([128, 256], F32, tag="u")
                nc.vector.scalar_tensor_tensor(
                    ub[:], tq[:], -0.5, ua[:], ALU.is_ge, ALU.add
                )
                uc = scr.tile([128, 256], F32, tag="u")
                nc.vector.scalar_tensor_tensor(
                    uc[:], tq[:], 0.5, ub[:], ALU.is_ge, ALU.add
                )
                renc = scr.tile([128, 256], BF16, tag="r")
                nc.vector.scalar_tensor_tensor(
                    renc[:], tq[:], 1.5, uc[:], ALU.is_lt, ALU.mult
                )
                for i in (0, 1):
                    eng = nc.sync if i % 2 == 0 else nc.scalar
                    eng.dma_start(
                        rc[r0 + i : r0 + i + 1, :],
                        renc[:, i * 128 : (i + 1) * 128],
                    )

            # phi broadcast for superchunk ci, pairs [plo,phi): one DMA per
            # pair; row 2p -> partitions 0:64, row 2p+1 -> 64:128
            def bcast(c0, clen=CHUNK, plo=0, phi=4, rrs=None):
                rrs = rrs if rrs is not None else [None] * 4
                for p in range(plo, phi):
                    rr = rrp.tile([128, CHUNK], BF16, tag="rr")
                    b = rc[2 * p : 2 * p + 2, c0 : c0 + clen]
                    src = bass.AP(
                        b.tensor, b.offset,
                        [list(b.ap[0]), [0, 64], [1, clen]],
                    )
                    eng = nc.sync if p % 2 == 0 else nc.scalar
                    eng.dma_start(rr[:, 0:clen], src)
                    rrs[p] = rr
                return rrs

            # p1 first: Pool's first multiply (phi*x of p1) heads the
            # serial Pool stream that paces the whole kernel
            encode_quarter(261, 1, 2)
            pipe = bcast(0, CHUNK, 1, 2)
            encode_quarter(0, 1, 0)
            bcast(0, CHUNK, 0, 1, pipe)
            encode_quarter(2, 130, 4)
            bcast(0, CHUNK, 2, 3, pipe)
            encode_quarter(130, 130, 6)
            bcast(0, CHUNK, 3, 4, pipe)
            slab_tiles[1] = load_slabs(1)

            # logical groups: 0=center; pair p slots 1+3p..3+3p = {f1,f2,f3}
            # f1 = phi*x; p0,p1: f2,f3 = E2*x, E3*x (DVE tensor_scalar);
            # p2,p3: f2,f3 = relu(2phi-3)*x, relu(2phi-5)*x (Act planes)
            # chunk plan: 7 full superchunks + 2 half chunks at the end
            # (halves shrink the post-last-build pipeline tail)
            chunks = [(i * CHUNK, CHUNK) for i in range(NCHUNK - 1)]
            b0 = (NCHUNK - 1) * CHUNK
            chunks += [(b0, 1024), (b0 + 1024, 512), (b0 + 1536, 512)]
            NITER = len(chunks)
            pending = None
            for ci in range(NITER):
                c0, clen = chunks[ci]
                reg = c0 // CHUNK
                rr = pipe
                xw = slab_tiles[reg]
                if ci + 1 < NITER:
                    pipe = bcast(*chunks[ci + 1])
                nreg = reg + 2
                if nreg < NCHUNK and nreg not in slab_tiles:
                    slab_tiles[nreg] = load_slabs(nreg)

                def xwin_of(p):
                    off = (0 if PAIR_SRC[p] == "A" else WIN) + 132
                    off += OFF[PAIRS[p][0]] + (c0 - reg * CHUNK)
                    return xw[:, off : off + clen]

                xms = {}

                # ---- Pool: phi*x for p1, p2, p3 (slow; consumed last) ----
                pool_ps = (1, 2) if ci == NITER - 1 else (1, 2, 3)
                for p in pool_ps:
                    xm = xmp.tile([128, CHUNK], BF16, tag="xm")
                    nc.gpsimd.tensor_tensor(
                        xm[:, 0:clen], rr[p][:, 0:clen], xwin_of(p), ALU.mult
                    )
                    xms[1 + 3 * p] = xm

                # ---- Act: relu basis planes: (p,j) in (1,2),(2,2),(2,3),
                # (3,2),(3,3); p1 f3 stays an E3 plane on DVE ----
                sg = {}
                for p, j, bv in (
                    (1, 2, b3), (2, 2, b3), (2, 3, b5), (3, 2, b3), (3, 3, b5)
                ):
                    s = sgp.tile([128, CHUNK], BF16, tag="sg")
                    nc.scalar.activation(
                        s[:, 0:clen], rr[p][:, 0:clen], AF.Relu,
                        bias=bv[:], scale=2.0,
                    )
                    sg[(p, j)] = s

                # previous chunk's PSUM -> SBUF copy + store, emitted after
                # this chunk's Act planes so it never delays them
                if pending is not None:
                    pacc, pc0, plen = pending
                    osb = outp.tile([O, CHUNK], BF16, tag="osb")
                    nc.scalar.activation(osb[:, 0:plen], pacc[:, 0:plen], AF.Copy)
                    nc.sync.dma_start(out_d[:, pc0 : pc0 + plen], osb[:, 0:plen])

                # ---- DVE: E planes + 9 multiplies ----
                def dve_tt(gid, a, p):
                    xm = xmp.tile([128, CHUNK], BF16, tag="xm")
                    nc.vector.tensor_tensor(
                        xm[:, 0:clen], a[:, 0:clen], xwin_of(p), ALU.mult
                    )
                    xms[gid] = xm

                # E planes first: DVE then streams multiplies gap-free at
                # a rate above PE consumption, so PE keeps a backlog
                ebs = {}
                for p, j in ((0, 2), (0, 3), (1, 3)):
                    eb = ebp.tile([128, CHUNK], BF16, tag="eb")
                    nc.vector.tensor_scalar(
                        eb[:, 0:clen], rr[p][:, 0:clen], j - 0.5, None, ALU.is_ge
                    )
                    ebs[(p, j)] = eb
                dve_tt(1, rr[0], 0)  # p0 f1 = phi*x
                dve_tt(2, ebs[(0, 2)], 0)
                dve_tt(3, ebs[(0, 3)], 0)
                dve_tt(6, ebs[(1, 3)], 1)
                dve_tt(5, sg[(1, 2)], 1)
                dve_tt(8, sg[(2, 2)], 2)
                dve_tt(9, sg[(2, 3)], 2)
                dve_tt(11, sg[(3, 2)], 3)
                dve_tt(12, sg[(3, 3)], 3)
                if ci == NITER - 1:
                    dve_tt(10, rr[3], 3)

                # ---- matmuls: 13 groups x PSUM banks ----
                # order: center first (free rhs), DVE tiles in build order,
                # Pool tiles last
                acc = psp.tile([O, CHUNK], F32)
                mm_rhs = [(0, xw[:, 132 + (c0 - reg * CHUNK) :
                               132 + (c0 - reg * CHUNK) + clen])]
                for gid in (1, 2, 3, 6, 5, 8, 9, 4, 11, 12, 7, 10):
                    mm_rhs.append((gid, xms[gid][:, 0:clen]))

                nmm = len(mm_rhs)
                MMN = 512
                for oi, (gid, rhs) in enumerate(mm_rhs):
                    for h in range(clen // MMN):
                        nc.tensor.matmul(
                            acc[:, h * MMN : (h + 1) * MMN],
                            wp[:, gid * 64 : (gid + 1) * 64],
                            bass.AP(
                                rhs.tensor,
                                rhs.offset + h * MMN,
                                [list(rhs.ap[0])] + [[1, MMN]],
                            ),
                            start=(oi == 0),
                            stop=(oi == nmm - 1),
                        )
                pending = (acc, c0, clen)

            pacc, pc0, plen = pending
            osb = outp.tile([O, CHUNK], BF16, tag="osb")
            nc.scalar.activation(osb[:, 0:plen], pacc[:, 0:plen], AF.Copy)
            nc.sync.dma_start(out_d[:, pc0 : pc0 + plen], osb[:, 0:plen])

    noop_cls = type(nc.sync.nop().ins)
    _split_excess_waits(nc, noop_cls, max_waits=1)
    return nc


def _bf(a):
    return a.astype(ml_dtypes.bfloat16).astype(np.float32)


def _prep_weights(w0, w1, w2):
    # basis-transformed weights; see module docstring.
    # p0 E-basis {phi,E2,E3}: V = [W2, W1-2W2, W0-W1-W2]
    # p1-p3 relu-basis {phi, relu(2phi-3), relu(2phi-5)} with values
    # {(1,2,3),(0,1,3),(0,0,1)} at phi=1,2,3: R = [W2, W1-2W2,
    # W0-3W1+3W2]  (p1 substitutes E3 for relu(2phi-5); same values)
    ws = [w0.reshape(O, C, 9), w1.reshape(O, C, 9), w2.reshape(O, C, 9)]
    W0, W1, W2 = ws
    wp = np.zeros((128, NGRP * 64), dtype=np.float32)

    def put(gi, ka, kb, arr):
        wp[0:64, gi * 64 : (gi + 1) * 64] = arr[:, :, ka].T
        if kb is not None:
            wp[64:128, gi * 64 : (gi + 1) * 64] = arr[:, :, kb].T

    put(0, 4, None, W1)  # center
    # E-basis (p0,p1) over {phi, E2, E3}; relu-basis (p2,p3) over
    # {phi, relu(2phi-3), relu(2phi-5)} = {(1,2,3),(0,1,3),(0,0,1)} at
    # phi=1,2,3 -> R = [W2, W1-2W2, W0-3W1+3W2]
    V = [W2, W1 - 2 * W2, W0 - W1 - W2]
    R = [W2, W1 - 2 * W2, W0 - 3 * W1 + 3 * W2]
    for p in range(4):
        ka, kb = PAIRS[p]
        basis = V if p == 0 else R
        for j in range(3):
            put(1 + p * 3 + j, ka, kb, basis[j])
    return wp.astype(ml_dtypes.bfloat16)


def _prep_dsh(dep, fxi):
    # [128, 518] f32: 3 x 130-col blocks of row-shifted depth (dh=-1,0,+1,
    # zero edges, 1-col left pad within each block) + cols 390:518 = fx/d
    d = np.zeros((128, 518), dtype=np.float32)
    d[1:128, 1:129] = dep[0:127]
    d[:, 131:259] = dep
    d[0:127, 261:389] = dep[1:128]
    d[:, 390:518] = fxi / dep
    return d


def _prep_x(x):
    # x: [C, L] f32 -> [128, 2*XW] bf16: tA = [x; x+1], tB = [x; x+128]
    xp = np.zeros((C, XW + 130), dtype=np.float32)
    xp[:, BASE : BASE + L] = x
    xt = np.zeros((128, 2 * XW), dtype=np.float32)
    xt[0:64, 0:XW] = xp[:, 0:XW]
    xt[64:128, 0:XW] = xp[:, 1 : XW + 1]
    xt[0:64, XW:] = xp[:, 0:XW]
    xt[64:128, XW:] = xp[:, 128 : XW + 128]
    return xt.astype(ml_dtypes.bfloat16)


def kernel(x, depth, fx, weight_0, weight_1, weight_2, _trace=False):
    x = np.asarray(x, dtype=np.float32)
    depth = np.asarray(depth, dtype=np.float32)
    fx = np.asarray(fx, dtype=np.float32)
    wp = _prep_weights(
        np.asarray(weight_0, np.float32),
        np.asarray(weight_1, np.float32),
        np.asarray(weight_2, np.float32),
    )
    in_maps = []
    for i in range(N_IMG):
        in_maps.append(
            {
                "xt": _prep_x(x[i].reshape(C, L)),
                "dshp": _prep_dsh(depth[i, 0], fx[i]),
                "wp": wp,
            }
        )
    nc = _build_graph()
    res = run_bass_kernel_spmd(nc, in_maps, core_ids=list(range(N_IMG)), trace=_trace)
    out = np.stack(
        [
            res.results[i]["out"].astype(np.float32).reshape(O, H, W)
            for i in range(N_IMG)
        ]
    )
    if _trace:
        return out, res
    return out


if __name__ == "__main__":
    rng = np.random.default_rng(0)
    ins = {
        "x": rng.standard_normal((N_IMG, C, H, W), dtype=np.float32),
        "depth": (1.0 + 9.0 * rng.random((N_IMG, 1, H, W))).astype(np.float32),
        "fx": (400.0 + 200.0 * rng.random(N_IMG)).astype(np.float32),
        "weight_0": rng.standard_normal((O, C, 3, 3), dtype=np.float32) * 0.04,
        "weight_1": rng.standard_normal((O, C, 3, 3), dtype=np.float32) * 0.04,
        "weight_2": rng.standard_normal((O, C, 3, 3), dtype=np.float32) * 0.04,
    }
    out = kernel(**ins)
    print("ran ok", out.shape, out.dtype)
